# revision 8
# baseline (speedup 1.0000x reference)
"""Grouped-Query Attention (S=2048, NQ=32, NKV=8, D=128, HID=4096) on 8 TRN2 NeuronCores.

Sharding: tensor-parallel over heads.  Core c owns KV head c and its 4
query heads (rows c*512..(c+1)*512 of Wq, c*128..(c+1)*128 of Wk/Wv, and
columns c*512..(c+1)*512 of Wo).  Each core computes a partial output
(row-parallel Wo); the host sums the 8 partials.

v2 schedule (everything fp16, fp32 PSUM accumulation):
  warmup -> K/V proj (all 4 seq chunks) -> per chunk t:
    Q-proj(t) block -> attention window B(t) with out-proj C(t-1)
    interleaved as PE filler -> final C(3).
  The PE never idles long enough for the HAM clock gate to re-throttle.

Softmax path per (head, chunk): scores are computed transposed
(S^T[j,i]), exp on ScalarE in [128,1024] two-key-tile batches (bf16 --
raw scores reach z~18, far beyond fp16 range), per-partition partial rowsums accumulated on DVE in
fp16 2x mode, cross-partition sum + broadcast via a ones-matmul,
reciprocal_approx_fast, one DVE mul to normalize ctx^T during PSUM
eviction.  Out-proj PSUM evictions alternate DVE/ScalarE; output is
written fp16 (host sums partials in fp32).

PSUM budget in the steady state window: scores 2x[128,1024] (4 banks,
shared with the Q-proj accumulators between windows) + ctx 2x[128,512]
(2) + outproj/rowsum 2x[128,512] (2) = 8 banks exactly.
"""

import os
import sys

import numpy as np

for _p in ("/opt/trn_rl_repo", "/root/.axon_site/_ro/trn_rl_repo"):
    if os.path.isdir(_p) and _p not in sys.path:
        sys.path.insert(0, _p)

import concourse.bass as bass
import concourse.bacc as bacc
import concourse.mybir as mybir
import concourse.tile as tile
from concourse.bass_utils import run_bass_kernel_spmd
from concourse.masks import make_identity

P = 128          # partitions / head dim / PE tile
S = 2048         # sequence length
HID = 4096       # hidden dim
NCORES = 8
NH = 4           # q heads per core
SC = 512         # free-dim chunk (PSUM bank = 512 fp32)
NKT = HID // P   # 32 contraction tiles over hidden
NCH = S // SC    # 4 sequence chunks
NJT = S // P     # 16 key tiles
NB = NJT // 2    # 8 two-key-tile batches
NOC = HID // SC  # 8 out column chunks
SCALE = float(P) ** -0.5
F16 = mybir.dt.float16
BF16 = mybir.dt.bfloat16
F32 = mybir.dt.float32
F16NP = np.dtype(np.float16)

_CACHE = {}


def _build():
    nc = bacc.Bacc(None, target_bir_lowering=False)
    x_p = nc.declare_dram_parameter("x_p", [P, NKT, S], F16, isOutput=False)
    wk_p = nc.declare_dram_parameter("wk_p", [P, NKT, P], F16, isOutput=False)
    wv_p = nc.declare_dram_parameter("wv_p", [P, NKT, P], F16, isOutput=False)
    bv_p = nc.declare_dram_parameter("bv_p", [P, 1], F32, isOutput=False)
    wq_p = nc.declare_dram_parameter("wq_p", [P, NKT, NH * P], F16, isOutput=False)
    wo_p = nc.declare_dram_parameter("wo_p", [P, NH, HID], F16, isOutput=False)
    out_p = nc.declare_dram_parameter("out_p", [(S // P) * NOC, P, SC], F16,
                                      isOutput=True)

    with tile.TileContext(nc) as tc:
        with (
            tc.tile_pool(name="consts", bufs=1) as consts,
            tc.tile_pool(name="acts", bufs=1) as acts,
            tc.tile_pool(name="xin", bufs=8) as xin,
            tc.tile_pool(name="epool", bufs=3) as epool,
            tc.tile_pool(name="rpool", bufs=2) as rpool,
            tc.tile_pool(name="rcpool", bufs=2) as rcpool,
            tc.tile_pool(name="opool", bufs=4) as opool,
        ):
            # ---- constants: identity + ones on-chip, weights via the
            # scalar-engine DMA queue so they never block x streaming ----
            wsrc = consts.tile([P, P], BF16)
            nc.vector.memset(wsrc, 1.0)
            ident = consts.tile([P, P], BF16)
            make_identity(nc, ident)
            ones16 = consts.tile([P, P], BF16)
            nc.gpsimd.memset(ones16, 1.0)
            bv_sb = consts.tile([P, 1], F32)
            nc.scalar.dma_start(out=bv_sb, in_=bv_p[:, :])
            wk = consts.tile([P, NKT, P], F16)
            wv = consts.tile([P, NKT, P], F16)
            for g in range(4):
                nc.scalar.dma_start(out=wk[:, g * 8:(g + 1) * 8],
                                    in_=wk_p[:, g * 8:(g + 1) * 8])
                nc.scalar.dma_start(out=wv[:, g * 8:(g + 1) * 8],
                                    in_=wv_p[:, g * 8:(g + 1) * 8])
            wq = consts.tile([P, NKT, NH * P], F16)
            wo = consts.tile([P, NH, HID], F16)

            # ---- persistent activations (fp16) ----
            qT = acts.tile([P, NH, S], F16)     # per head: [128 d, 2048 s]
            kT = acts.tile([P, S], F16)         # [128 d, 2048 s]
            vT = acts.tile([P, S], BF16)         # [128 d, 2048 s]
            v = acts.tile([P, NJT, P], BF16)     # [128 j, jt, 128 d]
            ctxT = acts.tile([P, NH, S], F16)   # per head: [128 d, 2048 i]

            # ---- PE warmup: release the HAM clock gate while the first
            # weight/x DMAs are in flight ----
            with tc.tile_pool(name="pwarm", bufs=1, space="PSUM") as pwarm:
                wt = pwarm.tile([P, P], BF16, name="warm")
                for _ in range(72):
                    nc.tensor.transpose(wt, wsrc, wsrc)

            # ---- phase KV: K/V projections for all chunks ----
            with tc.tile_pool(name="pkv", bufs=1, space="PSUM") as pkv:
                for c in range(NCH):
                    s0 = c * SC
                    k_ps = pkv.tile([P, SC], F32, tag="pk", bufs=2)
                    v_ps = pkv.tile([P, SC], F32, tag="pv", bufs=2)
                    for kt4 in range(0, NKT, 4):
                        xt = xin.tile([P, 4, SC], F16)
                        nc.sync.dma_start(out=xt, in_=x_p[:, kt4:kt4 + 4, s0:s0 + SC])
                        for j in range(4):
                            kt = kt4 + j
                            st, sp = kt == 0, kt == NKT - 1
                            nc.tensor.matmul(k_ps, lhsT=wk[:, kt, :], rhs=xt[:, j, :],
                                             start=st, stop=sp)
                            nc.tensor.matmul(v_ps, lhsT=wv[:, kt, :], rhs=xt[:, j, :],
                                             start=st, stop=sp)
                    if c == 1:
                        # wq is first needed at q-proj(0); fetching it here
                        # keeps early DMA bandwidth for wk/wv/x streaming
                        for g in range(4):
                            nc.scalar.dma_start(out=wq[:, g * 8:(g + 1) * 8],
                                                in_=wq_p[:, g * 8:(g + 1) * 8])
                    nc.scalar.copy(out=kT[:, s0:s0 + SC], in_=k_ps)
                    # v = x @ Wv.T + bv  (bias per-partition in [d, s] layout)
                    nc.scalar.activation(out=vT[:, s0:s0 + SC], in_=v_ps,
                                         func=mybir.ActivationFunctionType.Identity,
                                         bias=bv_sb, scale=1.0)
                # v[j, d] via PE transpose (vT chunks are all done or nearly so)
                for jt in range(NJT):
                    t_ps = pkv.tile([P, P], BF16, tag="ptr", bufs=2)
                    nc.tensor.transpose(t_ps, vT[:, jt * P:(jt + 1) * P], ident)
                    nc.vector.tensor_copy(out=v[:, jt, :], in_=t_ps)

            # ---- phases Q / B / C, software-pipelined ----
            with tc.tile_pool(name="pmain", bufs=1, space="PSUM") as pm:

                def emit_qproj(t):
                    s0 = t * SC
                    # share the 4 score banks: two [128,1024] allocations,
                    # each holding two heads' accumulators side by side
                    qa = pm.tile([P, 2 * SC], F32, tag="ps", bufs=2, name="qa")
                    qb = pm.tile([P, 2 * SC], F32, tag="ps", bufs=2, name="qb")
                    q_ps = [qa[:, 0:SC], qa[:, SC:2 * SC],
                            qb[:, 0:SC], qb[:, SC:2 * SC]]
                    for kt4 in range(0, NKT, 4):
                        xt = xin.tile([P, 4, SC], F16)
                        nc.sync.dma_start(out=xt, in_=x_p[:, kt4:kt4 + 4, s0:s0 + SC])
                        for j in range(4):
                            kt = kt4 + j
                            st, sp = kt == 0, kt == NKT - 1
                            for m in range(NH):
                                nc.tensor.matmul(q_ps[m],
                                                 lhsT=wq[:, kt, m * P:(m + 1) * P],
                                                 rhs=xt[:, j, :], start=st, stop=sp)
                    for m in range(NH):
                        nc.scalar.copy(out=qT[:, m, s0:s0 + SC], in_=q_ps[m])

                def make_cgroups(t):
                    # out-proj work for chunk t: 4 row tiles x 8 col chunks
                    return [(mt, oc) for mt in range(t * NCH, (t + 1) * NCH)
                            for oc in range(NOC)]

                def emit_cgroup(g, idx):
                    mt, oc = g
                    m0, o0 = mt * P, oc * SC
                    o_ps = pm.tile([P, SC], F32, tag="pout", bufs=2)
                    for dt_ in range(NH):
                        nc.tensor.matmul(o_ps, lhsT=ctxT[:, dt_, m0:m0 + P],
                                         rhs=wo[:, dt_, o0:o0 + SC],
                                         start=dt_ == 0, stop=dt_ == NH - 1)
                    ob = opool.tile([P, SC], F16)
                    if idx % 2 == 0:
                        nc.vector.tensor_copy(out=ob, in_=o_ps)
                    else:
                        nc.scalar.copy(out=ob, in_=o_ps)
                    nc.sync.dma_start(out=out_p[mt * NOC + oc], in_=ob)

                def emit_head_tail(t, tail):
                    th, tctx, tracc = tail
                    rb_ps = pm.tile([P, SC], F32, tag="pout", bufs=2, name="rb_ps")
                    nc.tensor.matmul(rb_ps, lhsT=ones16, rhs=tracc[:, 0:SC],
                                     start=True, stop=False)
                    nc.tensor.matmul(rb_ps, lhsT=ones16, rhs=tracc[:, SC:2 * SC],
                                     start=False, stop=True)
                    rbc = rcpool.tile([P, SC], F32, name="rbc")
                    nc.vector.reciprocal_approx_fast(out=rbc, in_=rb_ps)
                    nc.vector.tensor_mul(out=ctxT[:, th, t * SC:(t + 1) * SC],
                                         in0=tctx, in1=rbc)

                def emit_window(t, cgroups):
                    i0 = t * SC
                    ci = 0
                    prev = None       # (e, b, ctx_ps) pending lagged ctx MMs
                    tail = None       # (h, ctx_ps, racc) pending head tail
                    for h in range(NH):
                        ctx_ps = pm.tile([P, SC], F32, tag="pctx", bufs=2,
                                         name="ctx%d" % (h % 2))
                        racc = rpool.tile([P, 2 * SC], BF16, name="racc", bufs=2)
                        for b in range(NB):
                            # scores for key tiles 2b, 2b+1 (one PSUM tile)
                            s_ps = pm.tile([P, 2 * SC], F32, tag="ps", bufs=2)
                            j0 = 2 * b * P
                            nc.tensor.matmul(s_ps[:, 0:SC], lhsT=kT[:, j0:j0 + P],
                                             rhs=qT[:, h, i0:i0 + SC],
                                             start=True, stop=True)
                            nc.tensor.matmul(s_ps[:, SC:2 * SC],
                                             lhsT=kT[:, j0 + P:j0 + 2 * P],
                                             rhs=qT[:, h, i0:i0 + SC],
                                             start=True, stop=True)
                            e = epool.tile([P, 2 * SC], BF16)
                            nc.scalar.activation(out=e, in_=s_ps,
                                                 func=mybir.ActivationFunctionType.Exp,
                                                 scale=SCALE)
                            if b == 0:
                                nc.vector.tensor_copy(out=racc, in_=e)
                            else:
                                nc.vector.tensor_add(out=racc, in0=racc, in1=e)
                            # PE filler: one out-proj group of chunk t-1
                            if ci < len(cgroups):
                                emit_cgroup(cgroups[ci], ci)
                                ci += 1
                            # lagged ctx MMs for the previous exp batch
                            if prev is not None:
                                pe, pb, pctx = prev
                                jp = 2 * pb
                                nc.tensor.matmul(pctx, lhsT=v[:, jp, :],
                                                 rhs=pe[:, 0:SC],
                                                 start=pb == 0, stop=False)
                                nc.tensor.matmul(pctx, lhsT=v[:, jp + 1, :],
                                                 rhs=pe[:, SC:2 * SC],
                                                 start=False, stop=pb == NB - 1)
                            prev = (e, b, ctx_ps)
                            # previous head's rowsum/normalize, one slot late
                            if b == 1 and tail is not None:
                                emit_head_tail(t, tail)
                                tail = None
                        tail = (h, ctx_ps, racc)
                    # flush: last ctx MMs, last head tail, leftover C groups
                    pe, pb, pctx = prev
                    jp = 2 * pb
                    nc.tensor.matmul(pctx, lhsT=v[:, jp, :], rhs=pe[:, 0:SC],
                                     start=False, stop=False)
                    nc.tensor.matmul(pctx, lhsT=v[:, jp + 1, :], rhs=pe[:, SC:2 * SC],
                                     start=False, stop=True)
                    emit_head_tail(t, tail)
                    while ci < len(cgroups):
                        emit_cgroup(cgroups[ci], ci)
                        ci += 1

                emit_qproj(0)
                # wo is first needed in window 1; fetching it here keeps the
                # early DMA bandwidth for x/wk/wv/wq streaming
                for g in range(4):
                    nc.scalar.dma_start(out=wo[:, :, g * 1024:(g + 1) * 1024],
                                        in_=wo_p[:, :, g * 1024:(g + 1) * 1024])
                emit_window(0, [])
                for t in range(1, NCH):
                    emit_qproj(t)
                    emit_window(t, make_cgroups(t - 1))
                for ci, g in enumerate(make_cgroups(NCH - 1)):
                    emit_cgroup(g, ci)
    nc.finalize()
    return nc


def _get_program():
    if "nc" not in _CACHE:
        _CACHE["nc"] = _build()
    return _CACHE["nc"]


def _prep_inputs(hidden_states, Wq, Wk, Wv, bv, Wo):
    x = np.asarray(hidden_states, np.float32).reshape(S, HID)
    xT = np.ascontiguousarray(x.T).astype(F16NP)        # [HID, S]
    x_p = np.ascontiguousarray(
        xT.reshape(NKT, P, S).transpose(1, 0, 2))        # [P, NKT, S]
    Wq = np.asarray(Wq, np.float32)
    Wk = np.asarray(Wk, np.float32)
    Wv = np.asarray(Wv, np.float32)
    bv = np.asarray(bv, np.float32)
    Wo = np.asarray(Wo, np.float32)
    maps = []
    for c in range(NCORES):
        qs = slice(c * NH * P, (c + 1) * NH * P)
        ks = slice(c * P, (c + 1) * P)
        wq_t = Wq[qs].T.astype(F16NP)                    # [HID, 512]
        wq_prep = np.ascontiguousarray(
            wq_t.reshape(NKT, P, NH * P).transpose(1, 0, 2))
        wk_t = Wk[ks].T.astype(F16NP)                    # [HID, 128]
        wk_prep = np.ascontiguousarray(
            wk_t.reshape(NKT, P, P).transpose(1, 0, 2))
        wv_t = Wv[ks].T.astype(F16NP)
        wv_prep = np.ascontiguousarray(
            wv_t.reshape(NKT, P, P).transpose(1, 0, 2))
        wo_t = Wo[:, qs].T.astype(F16NP)                 # [512, HID]
        wo_prep = np.ascontiguousarray(
            wo_t.reshape(NH, P, HID).transpose(1, 0, 2))
        maps.append({
            "x_p": x_p,
            "wk_p": wk_prep,
            "wv_p": wv_prep,
            "bv_p": np.ascontiguousarray(bv[ks]).reshape(P, 1).astype(np.float32),
            "wq_p": wq_prep,
            "wo_p": wo_prep,
        })
    return maps


def kernel(hidden_states, Wq, Wk, Wv, bv, Wo, _trace=False, **kw):
    nc = _get_program()
    maps = _prep_inputs(hidden_states, Wq, Wk, Wv, bv, Wo)
    res = run_bass_kernel_spmd(nc, maps, list(range(NCORES)), trace=_trace, **kw)
    out = np.zeros((S, HID), np.float32)
    for c in range(NCORES):
        o = np.asarray(res.results[c]["out_p"], np.float32)  # [16*8, 128, 512]
        out += o.reshape(S // P, NOC, P, SC).transpose(0, 2, 1, 3).reshape(S, HID)
    if _trace:
        return out.reshape(1, S, HID), res
    return out.reshape(1, S, HID)
